# revision 1
# baseline (speedup 1.0000x reference)
"""Cross-modal contrastive loss on 8 Trainium2 NeuronCores.

Strategy (B=8192, d=256 hardcoded):
  * Host sorts rows by patient id, so the same-patient mask becomes a narrow
    diagonal band; casts projections to bf16.
  * Each core owns a 1024-row slice of z_a and the full z_t (pre-transposed to
    d-major on the host). It computes its (1024, 8192) block of
    exp(sim/T - C) with C a fixed upper bound on the logits (|sim|<=1 since
    rows are L2-normalized, so no running max is needed and partial sums
    combine by plain addition):
      - row sums via the ACT engine's accum_out (a2t direction),
      - column partial sums via a ones-vector matmul on the PE (t2a direction),
      - band (same-patient) row/col sums via a small extra matmul over a
        re-sliced z_t window plus a host-precomputed 0/1 mask.
  * Host combines the per-core partial sums, adds the positive (diagonal)
    terms, takes logs, and reduces to the scalar loss.
"""

import numpy as np
import ml_dtypes

TEMPERATURE = 0.03
SCALE = 1.0 / TEMPERATURE
C = SCALE + 0.01  # fixed logsumexp offset; logits are <= SCALE * (1 + eps)
B = 8192
D = 256
NCORES = 8
ROWS = B // NCORES          # 1024 rows per core
RT = ROWS // 128            # 8 row-tiles per core
NJ = B // 512               # 16 column chunks of 512
BF16 = ml_dtypes.bfloat16

_CACHE = {}


def _install_drain_patch():
    """walrus accepts at most one sync-wait per CTRL instruction, but
    TileContext's exit drain collects one wait per outstanding semaphore.
    Spread the waits across nop instructions, one wait each."""
    import bass_rust
    import concourse.tile as tile_mod
    from concourse.vector_clock import ScopedClock

    if getattr(tile_mod.TileContext, "_drain_patch_installed", False):
        return

    def _patched(self, tick_clock, wait_clock):
        nc = self.nc
        probe = nc.sync.nop(nofuse=True)
        wait_clock.add_sem_waits(
            probe.ins, ScopedClock({None: tick_clock.global_clock})
        )
        si = probe.ins.sync_info
        waits = list(si.on_wait) if si is not None else []
        if len(waits) > 1:
            si.on_wait = waits[:1]
            for w in waits[1:]:
                extra = nc.sync.nop(nofuse=True)
                extra.ins.sync_info = bass_rust.SyncInfo(on_wait=[w], on_update=[])
        nc.sync.drain()
        nc.all_engine_barrier()
        popped = nc._tile_sem_poison_stack.pop()
        assert popped is self._sem_poison
        nc.clear_and_free_semaphores(list(self.sems.allocated().values()))
        nc.all_engine_barrier()

    tile_mod.TileContext._drain_and_barrier = _patched
    tile_mod.TileContext._drain_patch_installed = True


def _split_multi_waits(nc):
    """walrus in this container accepts at most one sync-wait per instruction.
    Hoist extra waits onto same-engine nops inserted just before the
    instruction (engine streams are in-order, so the waits still gate it)."""
    import bass_rust

    n = 0
    for fn in nc.m.functions:
        for bb in fn.blocks:
            insts = list(bb.instructions)
            out = []
            for inst in insts:
                si = inst.sync_info
                if si is not None and len(si.on_wait) > 1:
                    waits = list(si.on_wait)
                    for w in waits[:-1]:
                        n += 1
                        nop = bass_rust.InstNoOp(
                            name=f"I-waitsplit-{n}", ins=[], outs=[]
                        )
                        nop.engine = inst.engine
                        nop.sync_info = bass_rust.SyncInfo(
                            on_wait=[w], on_update=[]
                        )
                        out.append(nop)
                    si.on_wait = waits[-1:]
                out.append(inst)
            if n:
                bb.instructions = out
    return n


def _build_program(PAD, W, Wb, split_waits=True):
    from contextlib import ExitStack
    import concourse.bass as bass
    import concourse.tile as tile
    from concourse import mybir

    _install_drain_patch()

    nc = bass.Bass()
    bf = mybir.dt.bfloat16
    f32 = mybir.dt.float32

    # Drop preamble memsets for const APs this program never uses (the
    # serial ~0.8us memsets sit ahead of the first DMA trigger).
    drop = ("const-float32-1.0", "const-bfloat16-1.0", "const-uint8-127")
    bb0 = nc.m.functions[0].blocks[0]
    bb0.instructions = [
        i for i in bb0.instructions
        if not (i.opcode == "Memset"
                and any(d in str(i.outs[0]) for d in drop))
    ]

    NB = 4          # column blocks of 2048 (4 chunks of 512 per psum tile)
    CPB = 4         # 512-chunks per block

    zaT = nc.declare_dram_parameter("zaT", [128, 2, ROWS], bf, isOutput=False)
    ztT = nc.declare_dram_parameter("ztT", [NB, 128, 2, CPB, 512], bf, isOutput=False)
    ztTb = nc.declare_dram_parameter("ztTb", [128, 2, Wb], bf, isOutput=False)
    maskb = nc.declare_dram_parameter("maskb", [128, RT, W], bf, isOutput=False)

    rowS = nc.declare_dram_parameter("rowS", [128, RT], f32, isOutput=True)
    bandrow = nc.declare_dram_parameter("bandrow", [128, RT], f32, isOutput=True)
    colP = nc.declare_dram_parameter("colP", [1, B], f32, isOutput=True)
    bandcol = nc.declare_dram_parameter("bandcol", [1, RT * W], f32, isOutput=True)

    with ExitStack() as ctx:
        tc = ctx.enter_context(tile.TileContext(nc))
        singles = ctx.enter_context(tc.tile_pool(name="singles", bufs=1))
        ztpool = ctx.enter_context(tc.tile_pool(name="ztpool", bufs=3))
        exppool = ctx.enter_context(tc.tile_pool(name="exppool", bufs=3))
        bpool = ctx.enter_context(tc.tile_pool(name="bpool", bufs=4))
        pairp = ctx.enter_context(tc.tile_pool(name="pairp", bufs=3))
        quadp = ctx.enter_context(tc.tile_pool(name="quadp", bufs=3))

        ones_sb = singles.tile([128, 1], bf)
        nc.vector.memset(ones_sb[:], 1.0)
        biasC = singles.tile([128, 1], f32)
        nc.vector.memset(biasC[:], -C)
        # Dummy activation: pull the ~1.3us exp table load off the critical
        # path (runs while the first DMAs are in flight).
        warm = singles.tile([128, 1], f32)
        nc.scalar.activation(warm[:], biasC[:], mybir.ActivationFunctionType.Exp)

        # DMA order matters: the band phase (first compute) needs only the
        # first row-tiles' weights and the first band window, so those land
        # first as separate tiles (dependencies are tile-granular). The mask
        # rides the SWDGE path (only needed once DVE touches it).
        zaT0 = singles.tile([128, 2, 256], bf)
        nc.sync.dma_start(zaT0[:], zaT[:, :, 0:256])
        ztTbp = []
        for p in range(RT // 2):
            t = singles.tile([128, 2, 384], bf, name=f"ztTbp{p}")
            nc.sync.dma_start(t[:], ztTb[:, :, 256 * p:256 * p + 384])
            ztTbp.append(t)
            if p == 0:
                zaT1 = singles.tile([128, 2, 768], bf)
                nc.sync.dma_start(zaT1[:], zaT[:, :, 256:ROWS])
        maskb_sb = singles.tile([128, RT, W], bf)
        nc.gpsimd.dma_start(maskb_sb[:], maskb[:])

        def za_sl(k, r):
            if r < 2:
                return zaT0[:, k, r * 128:(r + 1) * 128]
            return zaT1[:, k, (r - 2) * 128:(r - 1) * 128]

        acc = singles.tile([128, RT, NB], f32)
        rowS_sb = singles.tile([128, RT], f32)
        bandrow_sb = singles.tile([128, RT], f32)
        colS_sb = singles.tile([1, B], f32)
        bandcol_sb = singles.tile([1, RT * W], f32)
        colacc = [singles.tile([128, 2048], bf, name=f"colacc{b}")
                  for b in range(NB)]
        bandstack = singles.tile([128, RT, W], bf)

        # Main phase: (1024, 8192) block in four 2048-wide column blocks.
        # One weight load per (r, k) serves 4 chunk matmuls; one FD=2048 ACT
        # per (r, block) computes exp and the running row-sum (accum_out).
        # A DVE pairwise tree folds the 8 row-tiles' exp into colacc[b].
        pmain = ctx.enter_context(tc.tile_pool(name="pmain", bufs=2, space="PSUM"))
        # Band phase first: it only needs the small zaT/ztTb loads, so it
        # runs inside the DMA window while the big column blocks stream in.
        # sums over a W-wide window per row-tile, two row-tiles per pass.
        for r0 in range(0, RT, 2):
            pb = pmain.tile([128, 2048], f32, tag="pm")
            for k in range(2):
                for i in range(2):
                    nc.tensor.matmul(
                        pb[:, i * W:(i + 1) * W],
                        za_sl(k, r0 + i),
                        ztTbp[r0 // 2][:, k, i * 128:i * 128 + W],
                        start=(k == 0),
                        stop=(k == 1),
                        skip_group_check=True,
                    )
            exp_b = bpool.tile([128, 2 * W], bf, tag="exp_b")
            nc.scalar.activation(
                exp_b[:], pb[:, :2 * W], mybir.ActivationFunctionType.Exp,
                bias=biasC[:], scale=SCALE,
            )
            for i in range(2):
                nc.vector.scalar_tensor_tensor(
                    out=bandstack[:, r0 + i, :],
                    in0=exp_b[:, i * W:(i + 1) * W],
                    scalar=1.0,
                    in1=maskb_sb[:, r0 + i, :],
                    op0=mybir.AluOpType.mult,
                    op1=mybir.AluOpType.mult,
                    accum_out=bandrow_sb[:, r0 + i:r0 + i + 1],
                )

        if True:
            for b in range(NB):
                ztb0 = ztpool.tile([128, CPB, 512], bf, tag="ztb0")
                nc.sync.dma_start(ztb0[:], ztT[b, :, 0])
                ztb1 = ztpool.tile([128, CPB, 512], bf, tag="ztb1")
                nc.sync.dma_start(ztb1[:], ztT[b, :, 1])
                ztbk = (ztb0, ztb1)
                expts = {}
                pairs = []
                quads = []
                for r in range(RT):
                    pm = pmain.tile([128, 2048], f32, tag="pm")
                    for k in range(2):
                        for jj in range(CPB):
                            nc.tensor.matmul(
                                pm[:, jj * 512:(jj + 1) * 512],
                                za_sl(k, r),
                                ztbk[k][:, jj, :],
                                start=(k == 0),
                                stop=(k == 1),
                                skip_group_check=True,
                            )
                    ex = exppool.tile([128, 2048], bf, tag="exp")
                    nc.scalar.activation(
                        ex[:], pm[:], mybir.ActivationFunctionType.Exp,
                        bias=biasC[:], scale=SCALE,
                        accum_out=acc[:, r, b:b + 1],
                    )
                    expts[r] = ex
                    if r % 2 == 1:
                        p = pairp.tile([128, 2048], bf, tag="pair")
                        nc.vector.tensor_add(p[:], expts[r - 1][:], expts[r][:])
                        pairs.append(p)
                    if len(pairs) == 2:
                        q = quadp.tile([128, 2048], bf, tag="quad")
                        nc.vector.tensor_add(q[:], pairs[0][:], pairs[1][:])
                        quads.append(q)
                        pairs = []
                nc.vector.tensor_add(colacc[b][:], quads[0][:], quads[1][:])

            # Tail: partition-reduce colacc + bandstack via ones-matmuls.
            srcs = [(colacc[b][:], colS_sb[:, b * 2048:(b + 1) * 2048])
                    for b in range(NB)]
            srcs.append((bandstack[:].rearrange("p r w -> p (r w)"),
                         bandcol_sb[:]))
            for idx, (flat, dst) in enumerate(srcs):
                pcc = pmain.tile([1, 2048], f32, tag="pm")
                for jj in range(CPB):
                    nc.tensor.matmul(
                        pcc[0:1, jj * 512:(jj + 1) * 512], ones_sb[:],
                        flat[:, jj * 512:(jj + 1) * 512],
                        start=True, stop=True, skip_group_check=True,
                    )
                if idx % 2 == 0:
                    nc.scalar.copy(dst, pcc[:])
                else:
                    nc.vector.tensor_copy(dst, pcc[:])

        for r in range(RT):
            nc.vector.reduce_sum(
                out=rowS_sb[:, r:r + 1], in_=acc[:, r, :],
                axis=mybir.AxisListType.X,
            )
        nc.sync.dma_start(rowS[:], rowS_sb[:])
        nc.sync.dma_start(bandrow[:], bandrow_sb[:])
        nc.sync.dma_start(colP[:], colS_sb[:])
        nc.sync.dma_start(bandcol[:], bandcol_sb[:])

    if split_waits:
        _split_multi_waits(nc)
    return nc


def _prep_inputs(za16, zt16, pid_s, PAD, W, Wb):
    """Build the per-core input maps."""
    zt16T = np.ascontiguousarray(zt16.T)  # (256, 8192)
    ztT_all = np.ascontiguousarray(
        zt16T.reshape(2, 128, 4, 4, 512).transpose(2, 1, 0, 3, 4)
    )  # (NB, 128, 2, CPB, 512)

    pidp = np.full(B + 2 * PAD, -1, dtype=np.int64)
    pidp[PAD:PAD + B] = pid_s
    zt16T_pad = np.zeros((D, B + 2 * PAD), dtype=BF16)
    zt16T_pad[:, PAD:PAD + B] = zt16T

    in_maps = []
    for c in range(NCORES):
        r0 = c * ROWS
        zaTc = np.ascontiguousarray(
            za16[r0:r0 + ROWS].T.reshape(2, 128, ROWS).transpose(1, 0, 2)
        )  # (128, 2, ROWS)
        band = zt16T_pad[:, r0:r0 + Wb]  # global cols [r0-PAD, r0+Wb-PAD)
        ztTbc = np.ascontiguousarray(band.reshape(2, 128, Wb).transpose(1, 0, 2))
        mask = np.zeros((128, RT, W), dtype=BF16)
        for r in range(RT):
            rows = pid_s[r0 + r * 128: r0 + (r + 1) * 128]
            cols = pidp[r0 + r * 128: r0 + r * 128 + W]  # starts at global-PAD
            mask[:, r, :] = (rows[:, None] == cols[None, :]).astype(BF16)
        in_maps.append({"zaT": zaTc, "ztT": ztT_all, "ztTb": ztTbc, "maskb": mask})
    return in_maps


def _numpy_fallback(z_a, z_t, patient_ids):
    z_a = np.asarray(z_a, np.float64)
    z_t = np.asarray(z_t, np.float64)
    pid = np.asarray(patient_ids)
    sim = (z_a @ z_t.T) / TEMPERATURE
    cross = pid[:, None] != pid[None, :]

    def direction(sim, cross):
        n = sim.shape[0]
        pos = np.diagonal(sim)
        mask = cross | np.eye(n, dtype=bool)
        neg = np.where(mask, sim, -np.inf)
        m = neg.max(axis=1)
        lse = np.log(np.exp(neg - m[:, None]).sum(axis=1)) + m
        row_loss = lse - pos
        valid = cross.any(axis=1)
        cnt = valid.sum()
        return (row_loss[valid].sum() / cnt) if cnt > 0 else 0.0

    loss = 0.5 * (direction(sim, cross) + direction(sim.T, cross.T))
    return np.asarray(loss, dtype=np.float32)


def kernel(z_a, z_t, patient_ids):
    from concourse.bass_utils import run_bass_kernel_spmd

    z_a = np.asarray(z_a)
    z_t = np.asarray(z_t)
    pid = np.asarray(patient_ids)
    assert z_a.shape == (B, D) and z_t.shape == (B, D)

    # Sort rows by patient id so same-patient pairs live in a diagonal band.
    perm = np.argsort(pid, kind="stable")
    pid_s = pid[perm].astype(np.int64)
    za_s = z_a[perm]
    zt_s = z_t[perm]

    _, counts = np.unique(pid_s, return_counts=True)
    gmax = int(counts.max())
    if gmax > 64:
        return _numpy_fallback(z_a, z_t, patient_ids)
    PAD, W = 64, 256
    Wb = ROWS + 2 * PAD

    za16 = za_s.astype(BF16)
    zt16 = zt_s.astype(BF16)

    key = (PAD, W, Wb)
    if key not in _CACHE:
        _CACHE[key] = _build_program(PAD, W, Wb)
    nc = _CACHE[key]

    in_maps = _prep_inputs(za16, zt16, pid_s, PAD, W, Wb)
    r = run_bass_kernel_spmd(nc, in_maps, list(range(NCORES)))
    global _LAST_RESULT
    _LAST_RESULT = r
    res = r.results

    # Host-side assembly in float64.
    pos = (za16.astype(np.float64) * zt16.astype(np.float64)).sum(axis=1) * SCALE
    pos_exp = np.exp(pos - C)

    S_all = np.concatenate(
        [res[c]["rowS"].T.reshape(-1) for c in range(NCORES)]
    ).astype(np.float64)
    B_row = np.concatenate(
        [res[c]["bandrow"].T.reshape(-1) for c in range(NCORES)]
    ).astype(np.float64)
    colS = np.zeros(B, dtype=np.float64)
    for c in range(NCORES):
        colS += res[c]["colP"].reshape(-1).astype(np.float64)
    B_col = np.zeros(B, dtype=np.float64)
    for c in range(NCORES):
        bc = res[c]["bandcol"].reshape(RT, W).astype(np.float64)
        for r in range(RT):
            g0 = c * ROWS + r * 128 - PAD  # global col of band idx 0
            lo = max(0, -g0)
            hi = min(W, B - g0)
            B_col[g0 + lo:g0 + hi] += bc[r, lo:hi]

    Sa = np.maximum(S_all - B_row + pos_exp, 1e-300)
    St = np.maximum(colS - B_col + pos_exp, 1e-300)
    row_loss_a = C + np.log(Sa) - pos
    row_loss_t = C + np.log(St) - pos

    group_sizes = np.zeros(B, dtype=np.int64)
    uniq, inv, cnts = np.unique(pid_s, return_inverse=True, return_counts=True)
    group_sizes = cnts[inv]
    valid = group_sizes < B
    cnt = int(valid.sum())
    if cnt > 0:
        loss_a = row_loss_a[valid].sum() / cnt
        loss_t = row_loss_t[valid].sum() / cnt
    else:
        loss_a = loss_t = 0.0

    return np.asarray((loss_a + loss_t) / 2.0, dtype=np.float32)

